# revision 23
# baseline (speedup 1.0000x reference)
"""ANOVA-kernel (order 3) Trainium2 Bass kernel, v4 (segmented-scan DVE ops).

Math: per batch b, y[b] = sum_d e3(x[b, :, d]) with e3 the 3rd elementary
symmetric polynomial over F=64 fields. Newton's identities give

    e3 = (p1^3 - 3 p1 p2 + 2 p3) / 6,    p_k[b, d] = sum_f x[b, f, d]^k

Engine plan per [128 x 4096] fp16 tile (batch on partitions, free =
(h, d, f2) with h = f//32 major so both f-halves are contiguous):

  - p1[d], p2[d]: custom two-source segmented-scan DVE ops (SEGP1/SEGSQ)
    running in hardware 2X_1PORT mode: per cycle read 2 packed fp16 from
    each port (f-lo half on port0, f-hi on port1), accumulate in fp32,
    and write one (acc, acc) fp16 word per 32-element segment boundary
    (out_last_subdim). One 1127ns instruction replaces the old
    head-add + 5-level fold tree entirely.
  - p3: sin tiles use the Scalar engine (sum sin(x/8) = P1/8 - P3/3072 +
    O(t^5) with a free per-row accumulate); qc tiles build wide x^2/x^3
    with native 2x tensor_mul and fold x^3 per-d with SEGP1 (cube-style
    segmented custom ops hang the DVE - hardware-bisected). The sin/qc
    tile split balances ACT vs DVE occupancy; qc tiles sit early so the
    extra DVE work lands in Vector slack, not on the tail.
  - epilogue: ZW custom op computes (3 p2 - p1^2) p1 / -6 per d from the
    strided pair buffers, reduce over d, then per-tile-kind combine.

Inputs are downcast to fp16 on the host (norm-rel error ~2e-3, far under
the 2e-2 gate), halving HBM traffic.

Sharding: pure data parallel over batch across 8 NeuronCores.
"""

import numpy as np

_B, _F, _D = 8192, 64, 64
_NCORES = 8
_BP = _B // _NCORES     # batches per core
_P = 128                # partitions per tile
_FD = _F * _D           # free elems per batch
_H = _FD // 2

_T = _BP // _P          # tiles per core (8)
_N_QC = 1               # tiles computing p3 on DVE (SEGC); rest use ACT sin

# ---------------------------------------------------------------- custom ops

_OPS = {}


def _register(name, spec, uops, uops_2x=None, subdim=False):
    from concourse import dve_ops
    from concourse.dve_uop import DveOpSpec

    if name in _OPS:
        return _OPS[name]
    for op in dve_ops.OPS:
        if op.name == name:
            _OPS[name] = op
            return op
    row = 1 + len(dve_ops.OPS)
    shas = {}
    specs = {}
    for ver in ("v3", "v4"):
        dos = DveOpSpec(
            name=name, opcode=row, uops=uops, rd1_en=True,
            uops_2x=uops_2x if ver == "v3" else None,
        )
        specs[ver] = dos
        shas[ver] = dos.sha(ver)
    op = dve_ops.DveOp(name, spec, subdim=subdim, uops_sha=shas)
    dve_ops.OPS.append(op)
    dve_ops.CUSTOM_DVE_SPECS[name] = spec
    dve_ops._SUB_OPCODE_FOR_NAME[name] = row
    for ver in ("v3", "v4"):
        dve_ops._COMPILE_CACHE[(name, ver)] = specs[ver]
    return op


def _seg_fsm(u, state):
    """3-state FSM [seed, steady, step] for segmented-scan ops."""
    from concourse.dve_uop import Trigger

    if state == "seed":
        u.require_inp0 = 0
        u.require_inp1 = 0
        u.trigger = (Trigger.COUNT, Trigger.NONE, Trigger.NONE)
        u.repeat_count = 1
        u.next_uop = (1, 0, 0)
    elif state == "steady":
        u.require_inp0 = 1
        u.require_inp1 = 1
        u.trigger = (Trigger.SRC_TENSOR_DONE, Trigger.SUB_DIM_DONE, Trigger.NONE)
        u.next_uop = (0, 2, 0)
    else:  # step
        u.require_inp0 = 1
        u.require_inp1 = 1
        u.trigger = (Trigger.SRC_TENSOR_DONE, Trigger.SUB_DIM_DONE, Trigger.COUNT)
        u.next_uop = (0, 2, 1)
        u.repeat_count = 1
    return u


def _seg_out(u, state):
    from concourse.dve_uop import OutPath, OutSel

    if state != "seed":
        # full-word (dup) boundary write: half-word subdim writes hang in
        # 2x mode (hardware-verified), so emit (acc, acc) pairs
        u.enable_output(OutSel.ALU_OUT, OutPath.WR0_LO)
        u.enable_output(OutSel.ALU_OUT, OutPath.WR0_HI)
        u.out_last_subdim_enable = 1
    return u


def _mk_uops(build_one):
    return [build_one(s) for s in ("seed", "steady", "step")]


def _segp1_2x(state):
    """acc[seg] += a0+a1+b0+b1 (4 fp16/cycle); (acc,acc) at each boundary."""
    from concourse.dve_uop import AluInp, AluOp, DelayInp, InpSel, UopConfig

    D0, D1, D2, D3, D4 = (AluInp.PREV_DELAY_0, AluInp.PREV_DELAY_1,
                          AluInp.PREV_DELAY_2, AluInp.PREV_DELAY_3,
                          AluInp.PREV_DELAY_4)
    PREV, CURR = AluInp.PREV_ALU_OUT, AluInp.CURR_ALU_OUT
    u = UopConfig()
    u.enable_input(InpSel.SRC_0, 1)
    u.enable_input(InpSel.SRC_0_HI, 2)
    u.enable_input(InpSel.SRC_1, 3)
    u.enable_input(InpSel.SRC_1_HI, 4)
    u.enable_input(InpSel.ZERO, 5)
    d = u.datapath_config
    d[0].enable_alu(AluOp.ADD, D0, D1)
    d[0].pass_through_delay(2, 3, 4)
    d[1].enable_alu(AluOp.ADD, D2, D3)
    d[1].enable_delay_from_src(DelayInp.PREV_ALU_OUT, 0)
    d[1].pass_through_delay(4)
    d[2].enable_alu(AluOp.ADD, D0, PREV)
    d[2].pass_through_delay(4)
    if state == "steady":
        d[3].enable_alu(AluOp.ADD, CURR, PREV)
    elif state == "step":
        d[3].enable_alu(AluOp.ADD, D4, PREV)
    else:
        d[3].enable_alu(AluOp.BYPASS, D4, D4)
    for k in range(4, 8):
        d[k].pass_through_alu()
    return _seg_out(_seg_fsm(u, state), state)


def _segp1_1x(state):
    from concourse.dve_uop import AluInp, AluOp, InpSel, UopConfig

    D0, D1, D2 = (AluInp.PREV_DELAY_0, AluInp.PREV_DELAY_1,
                  AluInp.PREV_DELAY_2)
    PREV, CURR = AluInp.PREV_ALU_OUT, AluInp.CURR_ALU_OUT
    u = UopConfig()
    u.enable_input(InpSel.SRC_0, 1)
    u.enable_input(InpSel.SRC_1, 2)
    u.enable_input(InpSel.ZERO, 3)
    d = u.datapath_config
    d[0].enable_alu(AluOp.ADD, D0, D1)
    d[0].pass_through_delay(2)
    if state == "steady":
        d[1].enable_alu(AluOp.ADD, CURR, PREV)
    elif state == "step":
        d[1].enable_alu(AluOp.ADD, D2, PREV)
    else:
        d[1].enable_alu(AluOp.BYPASS, D2, D2)
    for k in range(2, 8):
        d[k].pass_through_alu()
    return _seg_out(_seg_fsm(u, state), state)


def _segsq_2x(state):
    """acc[seg] += a0^2+a1^2+b0^2+b1^2 (4 fp16/cycle)."""
    from concourse.dve_uop import AluInp, AluOp, DelayInp, InpSel, UopConfig

    D0, D1, D2, D3, D4 = (AluInp.PREV_DELAY_0, AluInp.PREV_DELAY_1,
                          AluInp.PREV_DELAY_2, AluInp.PREV_DELAY_3,
                          AluInp.PREV_DELAY_4)
    PREV, CURR = AluInp.PREV_ALU_OUT, AluInp.CURR_ALU_OUT
    u = UopConfig()
    u.enable_input(InpSel.SRC_0, 1)
    u.enable_input(InpSel.SRC_0_HI, 2)
    u.enable_input(InpSel.SRC_1, 3)
    u.enable_input(InpSel.SRC_1_HI, 4)
    u.enable_input(InpSel.ZERO, 5)
    d = u.datapath_config
    d[0].enable_alu(AluOp.MULTIPLY, D0, D0)
    d[0].pass_through_delay(1, 2, 3, 4)
    d[1].enable_alu(AluOp.MULTIPLY, D1, D1)
    d[1].enable_delay_from_src(DelayInp.PREV_ALU_OUT, 0)
    d[1].pass_through_delay(2, 3, 4)
    d[2].enable_alu(AluOp.MULTIPLY, D2, D2)
    d[2].enable_delay_from_src(DelayInp.PREV_ALU_OUT, 1)
    d[2].pass_through_delay(0, 3, 4)
    d[3].enable_alu(AluOp.MULTIPLY, D3, D3)
    d[3].enable_delay_from_src(DelayInp.PREV_ALU_OUT, 2)
    d[3].pass_through_delay(0, 1, 4)
    d[4].enable_alu(AluOp.ADD, D0, D1)
    d[4].enable_delay_from_src(DelayInp.PREV_ALU_OUT, 3)
    d[4].pass_through_delay(2, 4)
    d[5].enable_alu(AluOp.ADD, D2, D3)
    d[5].enable_delay_from_src(DelayInp.PREV_ALU_OUT, 0)
    d[5].pass_through_delay(4)
    d[6].enable_alu(AluOp.ADD, D0, PREV)
    d[6].pass_through_delay(4)
    if state == "steady":
        d[7].enable_alu(AluOp.ADD, CURR, PREV)
    elif state == "step":
        d[7].enable_alu(AluOp.ADD, D4, PREV)
    else:
        d[7].enable_alu(AluOp.BYPASS, D4, D4)
    return _seg_out(_seg_fsm(u, state), state)


def _segsq_1x(state):
    from concourse.dve_uop import AluInp, AluOp, DelayInp, InpSel, UopConfig

    D0, D1, D2 = (AluInp.PREV_DELAY_0, AluInp.PREV_DELAY_1,
                  AluInp.PREV_DELAY_2)
    PREV, CURR = AluInp.PREV_ALU_OUT, AluInp.CURR_ALU_OUT
    u = UopConfig()
    u.enable_input(InpSel.SRC_0, 1)
    u.enable_input(InpSel.SRC_1, 2)
    u.enable_input(InpSel.ZERO, 3)
    d = u.datapath_config
    d[0].enable_alu(AluOp.MULTIPLY, D0, D0)
    d[0].pass_through_delay(1, 2)
    d[1].enable_alu(AluOp.MULTIPLY, D1, D1)
    d[1].enable_delay_from_src(DelayInp.PREV_ALU_OUT, 0)
    d[1].pass_through_delay(2)
    d[2].enable_alu(AluOp.ADD, D0, PREV)
    d[2].pass_through_delay(2)
    if state == "steady":
        d[3].enable_alu(AluOp.ADD, CURR, PREV)
    elif state == "step":
        d[3].enable_alu(AluOp.ADD, D2, PREV)
    else:
        d[3].enable_alu(AluOp.BYPASS, D2, D2)
    for k in range(4, 8):
        d[k].pass_through_alu()
    return _seg_out(_seg_fsm(u, state), state)


def _segc_1x(state):
    """acc[seg] += a^3 + b^3 (1+1 fp16/cycle, fp32 accum)."""
    from concourse.dve_uop import AluInp, AluOp, DelayInp, InpSel, UopConfig

    D0, D1, D2, D3 = (AluInp.PREV_DELAY_0, AluInp.PREV_DELAY_1,
                      AluInp.PREV_DELAY_2, AluInp.PREV_DELAY_3)
    PREV, CURR = AluInp.PREV_ALU_OUT, AluInp.CURR_ALU_OUT
    u = UopConfig()
    u.enable_input(InpSel.SRC_0, 1)
    u.enable_input(InpSel.SRC_1, 2)
    u.enable_input(InpSel.ZERO, 3)
    d = u.datapath_config
    d[0].enable_alu(AluOp.MULTIPLY, D0, D0)               # a^2
    d[0].pass_through_delay(0, 1, 2)
    d[1].enable_alu(AluOp.MULTIPLY, PREV, D0)             # a^3
    d[1].pass_through_delay(1, 2)
    d[2].enable_alu(AluOp.MULTIPLY, D1, D1)               # b^2
    d[2].enable_delay_from_src(DelayInp.PREV_ALU_OUT, 3)  # c3 <- a^3
    d[2].pass_through_delay(1, 2)
    d[3].enable_alu(AluOp.MULTIPLY, PREV, D1)             # b^3
    d[3].pass_through_delay(2, 3)
    d[4].enable_alu(AluOp.ADD, D3, PREV)                  # a^3 + b^3
    d[4].pass_through_delay(2)
    if state == "steady":
        d[5].enable_alu(AluOp.ADD, CURR, PREV)
    elif state == "step":
        d[5].enable_alu(AluOp.ADD, D2, PREV)
    else:
        d[5].enable_alu(AluOp.BYPASS, D2, D2)
    for k in range(6, 8):
        d[k].pass_through_alu()
    return _seg_out(_seg_fsm(u, state), state)


def _segdot_2x(state):
    """acc[seg] += u0*v0 + u1*v1 (u on port0, v on port1, packed fp16)."""
    from concourse.dve_uop import AluInp, AluOp, DelayInp, InpSel, UopConfig

    D0, D1, D2, D3, D4 = (AluInp.PREV_DELAY_0, AluInp.PREV_DELAY_1,
                          AluInp.PREV_DELAY_2, AluInp.PREV_DELAY_3,
                          AluInp.PREV_DELAY_4)
    PREV, CURR = AluInp.PREV_ALU_OUT, AluInp.CURR_ALU_OUT
    u = UopConfig()
    u.enable_input(InpSel.SRC_0, 1)
    u.enable_input(InpSel.SRC_0_HI, 2)
    u.enable_input(InpSel.SRC_1, 3)
    u.enable_input(InpSel.SRC_1_HI, 4)
    u.enable_input(InpSel.ZERO, 5)
    d = u.datapath_config
    d[0].enable_alu(AluOp.MULTIPLY, D0, D2)
    d[0].pass_through_delay(1, 3, 4)
    d[1].enable_alu(AluOp.MULTIPLY, D1, D3)
    d[1].enable_delay_from_src(DelayInp.PREV_ALU_OUT, 0)
    d[1].pass_through_delay(4)
    d[2].enable_alu(AluOp.ADD, D0, PREV)
    d[2].pass_through_delay(4)
    if state == "steady":
        d[3].enable_alu(AluOp.ADD, CURR, PREV)
    elif state == "step":
        d[3].enable_alu(AluOp.ADD, D4, PREV)
    else:
        d[3].enable_alu(AluOp.BYPASS, D4, D4)
    for k in range(4, 8):
        d[k].pass_through_alu()
    return _seg_out(_seg_fsm(u, state), state)


def _segdot_1x(state):
    from concourse.dve_uop import AluInp, AluOp, InpSel, UopConfig

    D0, D1, D2 = (AluInp.PREV_DELAY_0, AluInp.PREV_DELAY_1,
                  AluInp.PREV_DELAY_2)
    PREV, CURR = AluInp.PREV_ALU_OUT, AluInp.CURR_ALU_OUT
    u = UopConfig()
    u.enable_input(InpSel.SRC_0, 1)
    u.enable_input(InpSel.SRC_1, 2)
    u.enable_input(InpSel.ZERO, 3)
    d = u.datapath_config
    d[0].enable_alu(AluOp.MULTIPLY, D0, D1)
    d[0].pass_through_delay(2)
    if state == "steady":
        d[1].enable_alu(AluOp.ADD, CURR, PREV)
    elif state == "step":
        d[1].enable_alu(AluOp.ADD, D2, PREV)
    else:
        d[1].enable_alu(AluOp.BYPASS, D2, D2)
    for k in range(2, 8):
        d[k].pass_through_alu()
    return _seg_out(_seg_fsm(u, state), state)


def _get_seg_ops():
    from concourse.dve_spec import Spec, Src0, Src1, scan, sq
    from concourse.dve_uop import AluOp

    def ref_p1(in0, in1, s0, s1, imm2):
        s = (in0.astype(np.float32) + in1.astype(np.float32)).sum(-1)
        return np.repeat(s[..., None], 2, axis=-1)

    def ref_sq(in0, in1, s0, s1, imm2):
        s = (in0.astype(np.float32) ** 2 + in1.astype(np.float32) ** 2).sum(-1)
        return np.repeat(s[..., None], 2, axis=-1)

    def ref_dot(in0, in1, s0, s1, imm2):
        s = (in0.astype(np.float32) * in1.astype(np.float32)).sum(-1)
        return np.repeat(s[..., None], 2, axis=-1)

    p1 = _register(
        "ANOVA_SEGP1", Spec(body=scan(AluOp.ADD, Src0 + Src1), reference=ref_p1),
        uops=_mk_uops(_segp1_1x), uops_2x=_mk_uops(_segp1_2x), subdim=True,
    )
    sq_ = _register(
        "ANOVA_SEGSQ",
        Spec(body=scan(AluOp.ADD, sq(Src0) + sq(Src1)), reference=ref_sq),
        uops=_mk_uops(_segsq_1x), uops_2x=_mk_uops(_segsq_2x), subdim=True,
    )
    dot = _register(
        "ANOVA_SEGDOT",
        Spec(body=scan(AluOp.ADD, Src0 * Src1), reference=ref_dot),
        uops=_mk_uops(_segdot_1x), uops_2x=_mk_uops(_segdot_2x), subdim=True,
    )
    return p1, sq_, dot


_ZW = [None]


def _get_zw_op():
    """w = (s0*p2 - p1^2) * p1 * s1 in one DVE pass (epilogue integrand)."""
    if _ZW[0] is not None:
        return _ZW[0]
    from concourse import dve_ops
    from concourse.dve_spec import C0, C1, Spec, Src0, Src1, lower, sq
    from concourse.dve_uop import DveOpSpec

    name = "ANOVA_ZW"
    for op in dve_ops.OPS:
        if op.name == name:
            _ZW[0] = op
            return op

    def _ref(in0, in1, s0, s1, imm2):
        p1 = in0.astype(np.float32)
        return (s0 * in1.astype(np.float32) - p1 * p1) * p1 * s1

    spec = Spec(body=(Src1 * C0 - sq(Src0)) * Src0 * C1, reference=_ref)
    row = 1 + len(dve_ops.OPS)
    shas = {}
    specs = {}
    for ver in ("v3", "v4"):
        dos = DveOpSpec(
            name=name, opcode=row, uops=lower(spec, ver=ver), rd1_en=True,
        )
        specs[ver] = dos
        shas[ver] = dos.sha(ver)
    op = dve_ops.DveOp(name, spec, subdim=False, uops_sha=shas)
    dve_ops.OPS.append(op)
    dve_ops.CUSTOM_DVE_SPECS[name] = spec
    dve_ops._SUB_OPCODE_FOR_NAME[name] = row
    for ver in ("v3", "v4"):
        dve_ops._COMPILE_CACHE[(name, ver)] = specs[ver]
    _ZW[0] = op
    return op


# ---------------------------------------------------------------- kernel


def build_nc(bp=_BP, n_qc=_N_QC):
    """Per-core Bass graph.

    Inputs:  "x"   [bp, 4096] fp16, free layout (h=f//32, d, f%32)
    Outputs: "out" [128, bp/128] f32, out[p, t] = y[t*128 + p]
    """
    from contextlib import ExitStack

    from concourse import bacc, mybir, tile

    f16 = mybir.dt.float16
    f32 = mybir.dt.float32
    AF = mybir.ActivationFunctionType
    OP = mybir.AluOpType
    AX = mybir.AxisListType

    segp1, segsq, segdot = _get_seg_ops()
    zw_op = _get_zw_op()

    T = bp // _P
    assert bp % _P == 0 and T % 2 == 0
    # qc tiles are the trailing tiles (so the ACT sin stream ends early),
    # but their DMA and DVE work are hoisted to the FRONT of the queues so
    # the extra Vector work lands in mid-stream slack, not on the tail
    qc_tiles = set(range(T - n_qc, T))
    qc_list = sorted(qc_tiles)

    nc = bacc.Bacc("TRN2", target_bir_lowering=False, debug=False)
    x_ext = nc.dram_tensor("x", [bp, _FD], f16, kind="ExternalInput").ap()
    y_ext = nc.dram_tensor("out", [_P, T], f32, kind="ExternalOutput").ap()

    with tile.TileContext(nc) as tc, ExitStack() as ctx:
        xp = ctx.enter_context(tc.tile_pool(name="x", bufs=7))
        sp = ctx.enter_context(tc.tile_pool(name="scr", bufs=2))
        pers = ctx.enter_context(tc.tile_pool(name="pers", bufs=1))

        # per-tile (value, value) fp16 pair buffers from the seg ops
        pb = pers.tile([_P, 2 * T * 2 * _D], f16, tag="pb")
        p1b = pb[:, :T * 2 * _D]            # (p1,p1) pairs, 128/tile
        p2b = pb[:, T * 2 * _D:]            # (p2,p2) pairs
        cb = pers.tile([_P, max(n_qc, 1) * 4 * _D], f16, tag="cb")
        sa1 = pers.tile([_P, T + 2], f32, tag="sa1")
        eacc = pers.tile([_P, T], f32, tag="eacc")
        p1f = pers.tile([_P, T], f32, tag="p1f")
        csum = pers.tile([_P, max(n_qc, 1)], f32, tag="csum")
        dq = pers.tile([_P, T], f32, tag="dq")
        out8 = pers.tile([_P, T], f32, tag="out8")
        er = pers.tile([_P, T * _D], f32, tag="er")

        xv_dram = x_ext.rearrange("(t p) q -> t p q", p=_P)

        # warm the Sin table during the first DMA wait (lazy load is ~2.6us)
        warm = pers.tile([_P, 1], f32, tag="warm")
        nc.gpsimd.memset(warm[:], 0.0)
        nc.scalar.activation(warm[:], warm[:], AF.Sin, scale=0.125)

        def seg(op, out_ap, in_lo, in_hi, pm=1):
            bi = nc.vector._custom_dve(
                op, out=out_ap,
                in0=in_lo.rearrange("p (s n) -> p s n", n=32),
                in1=in_hi.rearrange("p (s n) -> p s n", n=32),
            )
            bi.ins.perf_max = pm
            return bi

        def emit_heads(k, xt, nd, sacol, dcol):
            """Moment ops for a tile buffer xt holding nd d-groups (free
            nd*64, f-halves contiguous). Pair outputs land at column dcol
            (elements, 2 per d-group) of p1b/p2b (and cb for qc tiles)."""
            fd = nd * _F
            h = fd // 2
            xlo = xt[:, :h]
            xhi = xt[:, h:fd]
            seg(segp1, p1b[:, dcol:dcol + 2 * nd], xlo, xhi)
            seg(segsq, p2b[:, dcol:dcol + 2 * nd], xlo, xhi)
            if k in qc_tiles:
                # p3 on DVE with safe ops: wide x^2 via native 2x mul, then
                # a segmented dot sum(x^2 * x) per 32-seg, h-halves folded.
                qoff = qc_list.index(k) * 4 * _D
                x2t = sp.tile([_P, _FD], f16, tag="x2")
                nc.vector.tensor_mul(x2t[:, :fd], xt[:, :fd], xt[:, :fd])
                craw = cb[:, qoff:qoff + 4 * nd]
                bi = nc.vector._custom_dve(
                    segdot, out=craw,
                    in0=x2t[:, :fd].rearrange("p (s n) -> p s n", n=32),
                    in1=xt[:, :fd].rearrange("p (s n) -> p s n", n=32),
                )
                bi.ins.perf_max = 1
                nc.vector.tensor_add(
                    craw[:, :2 * nd], craw[:, :2 * nd], craw[:, 2 * nd:]
                )
            else:
                sscr = sp.tile([_P, _FD], f16, tag="sscr")
                nc.scalar.activation(
                    sscr[:, :fd], xt[:, :fd], AF.Sin, scale=0.125,
                    accum_out=sa1[:, sacol:sacol + 1],
                )

        def pairs_lo(buf, c0, c1):
            """[p, n, 1] view of the first element of each fp16 pair in
            buf[:, 2*c0 : 2*c1]."""
            return buf[:, 2 * c0:2 * c1].rearrange(
                "p (n two) -> p n two", two=2
            )[:, :, 0:1]

        def eruns(lo, hi):
            """Split [lo, hi) into kind-contiguous tile runs."""
            runs = []
            s = lo
            for c in range(lo + 1, hi + 1):
                if c == hi or (c in qc_tiles) != (s in qc_tiles):
                    runs.append((s, c))
                    s = c
            return runs

        def epilogue(c0, c1):
            """e3 combine for tile-columns [c0, c1) (uniform kind)."""
            if c0 >= c1:
                return
            is_qc = c0 in qc_tiles
            nt = c1 - c0
            d0, d1 = c0 * _D, c1 * _D
            nc.vector._custom_dve(
                zw_op, out=er[:, d0:d1],
                in0=pairs_lo(p1b, d0, d1),
                in1=pairs_lo(p2b, d0, d1),
                s0=3.0, s1=-1.0 / 6.0,
            )
            nc.vector.reduce_sum(
                eacc[:, c0:c1],
                er[:, d0:d1].rearrange("p (t d) -> p t d", t=nt, d=_D),
                axis=AX.X,
            )
            if is_qc:
                q0 = qc_list.index(c0)
                nc.vector.reduce_sum(
                    csum[:, q0:q0 + nt],
                    cb[:, 4 * q0 * _D:4 * q0 * _D + nt * 4 * _D].rearrange(
                        "p (t four) -> p t four", t=nt
                    )[:, :, :2 * _D],
                    axis=AX.X,
                )
                # out = eacc + (1/3) * (csum/2)   (pairs double-count c)
                nc.vector.scalar_tensor_tensor(
                    out8[:, c0:c1], csum[:, q0:q0 + nt], 1.0 / 6.0,
                    eacc[:, c0:c1], OP.mult, OP.add,
                )
            else:
                nc.vector.reduce_sum(
                    p1f[:, c0:c1],
                    pairs_lo(p1b, d0, d1).rearrange(
                        "p (t d) one -> p t (d one)", t=nt),
                    axis=AX.X,
                )
                # out = eacc + 128 p1f - 1024 S1   (P3 = 384 p1f - 3072 S1)
                nc.vector.scalar_tensor_tensor(
                    dq[:, c0:c1], sa1[:, c0:c1], -1024.0, eacc[:, c0:c1],
                    OP.mult, OP.add,
                )
                nc.vector.scalar_tensor_tensor(
                    out8[:, c0:c1], p1f[:, c0:c1], 128.0, dq[:, c0:c1],
                    OP.mult, OP.add,
                )

        # tile0 pieces first (unblocks both engines fastest); piece a is
        # tiny (d0-1) so the first sin issues as soon as the table loads
        xta = sp.tile([_P, 1024], f16, tag="xta")
        nc.sync.dma_start(xta[:, :128], xv_dram[0][:, :128])
        nc.sync.dma_start(xta[:, 128:], xv_dram[0][:, 128:1024])
        xtb = sp.tile([_P, 3072], f16, tag="xtb")
        nc.sync.dma_start(xtb[:], xv_dram[0][:, 1024:])
        # qc-tile DMA goes mid-stream: at the front it starves the early
        # sins, at the back its DVE burst extends the Vector tail (both
        # measured); mid-queue it overlaps the sin stream on both sides
        sins = [k for k in range(1, T) if k not in qc_tiles]
        order = sins[:3] + qc_list + sins[3:]
        xts = {}
        for k in order:
            xts[k] = xp.tile([_P, _FD], f16, tag="xt", name=f"xt{k}")
            nc.sync.dma_start(xts[k][:], xv_dram[k])

        emit_heads(0, xta[:, :128], 2, T, 0)
        emit_heads(0, xta[:, 128:], 14, T + 1, 4)
        emit_heads(0, xtb, 48, 0, 32)
        nc.vector.scalar_tensor_tensor(
            sa1[:, 0:1], sa1[:, T:T + 1], 1.0, sa1[:, 0:1],
            OP.mult, OP.add,
        )
        nc.vector.scalar_tensor_tensor(
            sa1[:, 0:1], sa1[:, T + 1:T + 2], 1.0, sa1[:, 0:1],
            OP.mult, OP.add,
        )
        for k in order:
            emit_heads(k, xts[k], _D, k, k * 2 * _D)
            if k == T // 2 + 1:
                for a, b in eruns(0, T // 2):
                    epilogue(a, b)
                nc.sync.dma_start(y_ext[:, :T // 2], out8[:, :T // 2])
        for a, b in eruns(T // 2, T):
            epilogue(a, b)
        nc.sync.dma_start(y_ext[:, T // 2:], out8[:, T // 2:])

    nc.compile()
    return nc


_nc_cache = {}


def _get_nc():
    key = (_BP, _N_QC)
    if key not in _nc_cache:
        _nc_cache[key] = build_nc(*key)
    return _nc_cache[key]


def _marshal(x: np.ndarray) -> list:
    """FULL fp32 input [B, F, D] -> per-core fp16 arrays [bp, 4096] in
    (tile-internal) layout: per batch, free = (h=f//32, d, f%32), with
    tile 0 of each core d-split into two self-contained halves."""
    x = np.asarray(x)
    assert x.shape == (_B, _F, _D), x.shape
    xc = x.reshape(_NCORES, _BP, _F, _D).astype(np.float16)
    xt = xc.reshape(_NCORES, _BP, 2, 32, _D).transpose(0, 1, 2, 4, 3)
    out = np.empty((_NCORES, _BP, _FD), dtype=np.float16)
    flat = xt.reshape(_NCORES, _BP, _FD)
    out[:, _P:] = flat[:, _P:]
    # tile 0 pieces: d 0-1 (tiny, so ACT/DVE start right after the sin
    # table loads), d 2-15, d 16-63; each piece h-major so its f-halves
    # are contiguous and self-contained
    t0 = xt[:, :_P]                                   # [c, 128, h, d, f2]
    pa = t0[:, :, :, 0:2].reshape(_NCORES, _P, 128)
    pb = t0[:, :, :, 2:16].reshape(_NCORES, _P, 896)
    pc = t0[:, :, :, 16:64].reshape(_NCORES, _P, 3072)
    out[:, :_P] = np.concatenate([pa, pb, pc], axis=2)
    return [np.ascontiguousarray(out[c]) for c in range(_NCORES)]


def kernel(x: np.ndarray) -> np.ndarray:
    from concourse.bass_utils import run_bass_kernel_spmd

    nc = _get_nc()
    shards = _marshal(x)
    in_maps = [{"x": shards[c]} for c in range(_NCORES)]
    res = run_bass_kernel_spmd(nc, in_maps, core_ids=list(range(_NCORES)))
    outs = []
    for c in range(_NCORES):
        o = res.results[c]["out"]  # [128, T]; o[p, t] = y[t*128 + p]
        outs.append(np.asarray(o).T.reshape(-1))
    return np.concatenate(outs).reshape(_B, 1).astype(np.float32)


# revision 24
# speedup vs baseline: 1.0297x; 1.0297x over previous
"""ANOVA-kernel (order 3) Trainium2 Bass kernel, v4 (segmented-scan DVE ops).

Math: per batch b, y[b] = sum_d e3(x[b, :, d]) with e3 the 3rd elementary
symmetric polynomial over F=64 fields. Newton's identities give

    e3 = (p1^3 - 3 p1 p2 + 2 p3) / 6,    p_k[b, d] = sum_f x[b, f, d]^k

Engine plan per [128 x 4096] fp16 tile (batch on partitions, free =
(h, d, f2) with h = f//32 major so both f-halves are contiguous):

  - p1[d], p2[d]: custom two-source segmented-scan DVE ops (SEGP1/SEGSQ)
    running in hardware 2X_1PORT mode: per cycle read 2 packed fp16 from
    each port (f-lo half on port0, f-hi on port1), accumulate in fp32,
    and write one (acc, acc) fp16 word per 32-element segment boundary
    (out_last_subdim). One 1127ns instruction replaces the old
    head-add + 5-level fold tree entirely.
  - p3: sin tiles use the Scalar engine (sum sin(x/8) = P1/8 - P3/3072 +
    O(t^5) with a free per-row accumulate); qc tiles build wide x^2/x^3
    with native 2x tensor_mul and fold x^3 per-d with SEGP1 (cube-style
    segmented custom ops hang the DVE - hardware-bisected). The sin/qc
    tile split balances ACT vs DVE occupancy; qc tiles sit early so the
    extra DVE work lands in Vector slack, not on the tail.
  - epilogue: ZW custom op computes (3 p2 - p1^2) p1 / -6 per d from the
    strided pair buffers, reduce over d, then per-tile-kind combine.

Inputs are downcast to fp16 on the host (norm-rel error ~2e-3, far under
the 2e-2 gate), halving HBM traffic.

Sharding: pure data parallel over batch across 8 NeuronCores.
"""

import numpy as np

_B, _F, _D = 8192, 64, 64
_NCORES = 8
_BP = _B // _NCORES     # batches per core
_P = 128                # partitions per tile
_FD = _F * _D           # free elems per batch
_H = _FD // 2

_T = _BP // _P          # tiles per core (8)
_N_QC = 1               # tiles computing p3 on DVE (SEGC); rest use ACT sin

# ---------------------------------------------------------------- custom ops

_OPS = {}


def _register(name, spec, uops, uops_2x=None, subdim=False):
    from concourse import dve_ops
    from concourse.dve_uop import DveOpSpec

    if name in _OPS:
        return _OPS[name]
    for op in dve_ops.OPS:
        if op.name == name:
            _OPS[name] = op
            return op
    row = 1 + len(dve_ops.OPS)
    shas = {}
    specs = {}
    for ver in ("v3", "v4"):
        dos = DveOpSpec(
            name=name, opcode=row, uops=uops, rd1_en=True,
            uops_2x=uops_2x if ver == "v3" else None,
        )
        specs[ver] = dos
        shas[ver] = dos.sha(ver)
    op = dve_ops.DveOp(name, spec, subdim=subdim, uops_sha=shas)
    dve_ops.OPS.append(op)
    dve_ops.CUSTOM_DVE_SPECS[name] = spec
    dve_ops._SUB_OPCODE_FOR_NAME[name] = row
    for ver in ("v3", "v4"):
        dve_ops._COMPILE_CACHE[(name, ver)] = specs[ver]
    return op


def _seg_fsm(u, state):
    """3-state FSM [seed, steady, step] for segmented-scan ops."""
    from concourse.dve_uop import Trigger

    if state == "seed":
        u.require_inp0 = 0
        u.require_inp1 = 0
        u.trigger = (Trigger.COUNT, Trigger.NONE, Trigger.NONE)
        u.repeat_count = 1
        u.next_uop = (1, 0, 0)
    elif state == "steady":
        u.require_inp0 = 1
        u.require_inp1 = 1
        u.trigger = (Trigger.SRC_TENSOR_DONE, Trigger.SUB_DIM_DONE, Trigger.NONE)
        u.next_uop = (0, 2, 0)
    else:  # step
        u.require_inp0 = 1
        u.require_inp1 = 1
        u.trigger = (Trigger.SRC_TENSOR_DONE, Trigger.SUB_DIM_DONE, Trigger.COUNT)
        u.next_uop = (0, 2, 1)
        u.repeat_count = 1
    return u


def _seg_out(u, state):
    from concourse.dve_uop import OutPath, OutSel

    if state != "seed":
        # full-word (dup) boundary write: half-word subdim writes hang in
        # 2x mode (hardware-verified), so emit (acc, acc) pairs
        u.enable_output(OutSel.ALU_OUT, OutPath.WR0_LO)
        u.enable_output(OutSel.ALU_OUT, OutPath.WR0_HI)
        u.out_last_subdim_enable = 1
    return u


def _mk_uops(build_one):
    return [build_one(s) for s in ("seed", "steady", "step")]


def _segp1_2x(state):
    """acc[seg] += a0+a1+b0+b1 (4 fp16/cycle); (acc,acc) at each boundary."""
    from concourse.dve_uop import AluInp, AluOp, DelayInp, InpSel, UopConfig

    D0, D1, D2, D3, D4 = (AluInp.PREV_DELAY_0, AluInp.PREV_DELAY_1,
                          AluInp.PREV_DELAY_2, AluInp.PREV_DELAY_3,
                          AluInp.PREV_DELAY_4)
    PREV, CURR = AluInp.PREV_ALU_OUT, AluInp.CURR_ALU_OUT
    u = UopConfig()
    u.enable_input(InpSel.SRC_0, 1)
    u.enable_input(InpSel.SRC_0_HI, 2)
    u.enable_input(InpSel.SRC_1, 3)
    u.enable_input(InpSel.SRC_1_HI, 4)
    u.enable_input(InpSel.ZERO, 5)
    d = u.datapath_config
    d[0].enable_alu(AluOp.ADD, D0, D1)
    d[0].pass_through_delay(2, 3, 4)
    d[1].enable_alu(AluOp.ADD, D2, D3)
    d[1].enable_delay_from_src(DelayInp.PREV_ALU_OUT, 0)
    d[1].pass_through_delay(4)
    d[2].enable_alu(AluOp.ADD, D0, PREV)
    d[2].pass_through_delay(4)
    if state == "steady":
        d[3].enable_alu(AluOp.ADD, CURR, PREV)
    elif state == "step":
        d[3].enable_alu(AluOp.ADD, D4, PREV)
    else:
        d[3].enable_alu(AluOp.BYPASS, D4, D4)
    for k in range(4, 8):
        d[k].pass_through_alu()
    return _seg_out(_seg_fsm(u, state), state)


def _segp1_1x(state):
    from concourse.dve_uop import AluInp, AluOp, InpSel, UopConfig

    D0, D1, D2 = (AluInp.PREV_DELAY_0, AluInp.PREV_DELAY_1,
                  AluInp.PREV_DELAY_2)
    PREV, CURR = AluInp.PREV_ALU_OUT, AluInp.CURR_ALU_OUT
    u = UopConfig()
    u.enable_input(InpSel.SRC_0, 1)
    u.enable_input(InpSel.SRC_1, 2)
    u.enable_input(InpSel.ZERO, 3)
    d = u.datapath_config
    d[0].enable_alu(AluOp.ADD, D0, D1)
    d[0].pass_through_delay(2)
    if state == "steady":
        d[1].enable_alu(AluOp.ADD, CURR, PREV)
    elif state == "step":
        d[1].enable_alu(AluOp.ADD, D2, PREV)
    else:
        d[1].enable_alu(AluOp.BYPASS, D2, D2)
    for k in range(2, 8):
        d[k].pass_through_alu()
    return _seg_out(_seg_fsm(u, state), state)


def _segsq_2x(state):
    """acc[seg] += a0^2+a1^2+b0^2+b1^2 (4 fp16/cycle)."""
    from concourse.dve_uop import AluInp, AluOp, DelayInp, InpSel, UopConfig

    D0, D1, D2, D3, D4 = (AluInp.PREV_DELAY_0, AluInp.PREV_DELAY_1,
                          AluInp.PREV_DELAY_2, AluInp.PREV_DELAY_3,
                          AluInp.PREV_DELAY_4)
    PREV, CURR = AluInp.PREV_ALU_OUT, AluInp.CURR_ALU_OUT
    u = UopConfig()
    u.enable_input(InpSel.SRC_0, 1)
    u.enable_input(InpSel.SRC_0_HI, 2)
    u.enable_input(InpSel.SRC_1, 3)
    u.enable_input(InpSel.SRC_1_HI, 4)
    u.enable_input(InpSel.ZERO, 5)
    d = u.datapath_config
    d[0].enable_alu(AluOp.MULTIPLY, D0, D0)
    d[0].pass_through_delay(1, 2, 3, 4)
    d[1].enable_alu(AluOp.MULTIPLY, D1, D1)
    d[1].enable_delay_from_src(DelayInp.PREV_ALU_OUT, 0)
    d[1].pass_through_delay(2, 3, 4)
    d[2].enable_alu(AluOp.MULTIPLY, D2, D2)
    d[2].enable_delay_from_src(DelayInp.PREV_ALU_OUT, 1)
    d[2].pass_through_delay(0, 3, 4)
    d[3].enable_alu(AluOp.MULTIPLY, D3, D3)
    d[3].enable_delay_from_src(DelayInp.PREV_ALU_OUT, 2)
    d[3].pass_through_delay(0, 1, 4)
    d[4].enable_alu(AluOp.ADD, D0, D1)
    d[4].enable_delay_from_src(DelayInp.PREV_ALU_OUT, 3)
    d[4].pass_through_delay(2, 4)
    d[5].enable_alu(AluOp.ADD, D2, D3)
    d[5].enable_delay_from_src(DelayInp.PREV_ALU_OUT, 0)
    d[5].pass_through_delay(4)
    d[6].enable_alu(AluOp.ADD, D0, PREV)
    d[6].pass_through_delay(4)
    if state == "steady":
        d[7].enable_alu(AluOp.ADD, CURR, PREV)
    elif state == "step":
        d[7].enable_alu(AluOp.ADD, D4, PREV)
    else:
        d[7].enable_alu(AluOp.BYPASS, D4, D4)
    return _seg_out(_seg_fsm(u, state), state)


def _segsq_1x(state):
    from concourse.dve_uop import AluInp, AluOp, DelayInp, InpSel, UopConfig

    D0, D1, D2 = (AluInp.PREV_DELAY_0, AluInp.PREV_DELAY_1,
                  AluInp.PREV_DELAY_2)
    PREV, CURR = AluInp.PREV_ALU_OUT, AluInp.CURR_ALU_OUT
    u = UopConfig()
    u.enable_input(InpSel.SRC_0, 1)
    u.enable_input(InpSel.SRC_1, 2)
    u.enable_input(InpSel.ZERO, 3)
    d = u.datapath_config
    d[0].enable_alu(AluOp.MULTIPLY, D0, D0)
    d[0].pass_through_delay(1, 2)
    d[1].enable_alu(AluOp.MULTIPLY, D1, D1)
    d[1].enable_delay_from_src(DelayInp.PREV_ALU_OUT, 0)
    d[1].pass_through_delay(2)
    d[2].enable_alu(AluOp.ADD, D0, PREV)
    d[2].pass_through_delay(2)
    if state == "steady":
        d[3].enable_alu(AluOp.ADD, CURR, PREV)
    elif state == "step":
        d[3].enable_alu(AluOp.ADD, D2, PREV)
    else:
        d[3].enable_alu(AluOp.BYPASS, D2, D2)
    for k in range(4, 8):
        d[k].pass_through_alu()
    return _seg_out(_seg_fsm(u, state), state)


def _segc_1x(state):
    """acc[seg] += a^3 + b^3 (1+1 fp16/cycle, fp32 accum)."""
    from concourse.dve_uop import AluInp, AluOp, DelayInp, InpSel, UopConfig

    D0, D1, D2, D3 = (AluInp.PREV_DELAY_0, AluInp.PREV_DELAY_1,
                      AluInp.PREV_DELAY_2, AluInp.PREV_DELAY_3)
    PREV, CURR = AluInp.PREV_ALU_OUT, AluInp.CURR_ALU_OUT
    u = UopConfig()
    u.enable_input(InpSel.SRC_0, 1)
    u.enable_input(InpSel.SRC_1, 2)
    u.enable_input(InpSel.ZERO, 3)
    d = u.datapath_config
    d[0].enable_alu(AluOp.MULTIPLY, D0, D0)               # a^2
    d[0].pass_through_delay(0, 1, 2)
    d[1].enable_alu(AluOp.MULTIPLY, PREV, D0)             # a^3
    d[1].pass_through_delay(1, 2)
    d[2].enable_alu(AluOp.MULTIPLY, D1, D1)               # b^2
    d[2].enable_delay_from_src(DelayInp.PREV_ALU_OUT, 3)  # c3 <- a^3
    d[2].pass_through_delay(1, 2)
    d[3].enable_alu(AluOp.MULTIPLY, PREV, D1)             # b^3
    d[3].pass_through_delay(2, 3)
    d[4].enable_alu(AluOp.ADD, D3, PREV)                  # a^3 + b^3
    d[4].pass_through_delay(2)
    if state == "steady":
        d[5].enable_alu(AluOp.ADD, CURR, PREV)
    elif state == "step":
        d[5].enable_alu(AluOp.ADD, D2, PREV)
    else:
        d[5].enable_alu(AluOp.BYPASS, D2, D2)
    for k in range(6, 8):
        d[k].pass_through_alu()
    return _seg_out(_seg_fsm(u, state), state)


def _segdot_2x(state):
    """acc[seg] += u0*v0 + u1*v1 (u on port0, v on port1, packed fp16)."""
    from concourse.dve_uop import AluInp, AluOp, DelayInp, InpSel, UopConfig

    D0, D1, D2, D3, D4 = (AluInp.PREV_DELAY_0, AluInp.PREV_DELAY_1,
                          AluInp.PREV_DELAY_2, AluInp.PREV_DELAY_3,
                          AluInp.PREV_DELAY_4)
    PREV, CURR = AluInp.PREV_ALU_OUT, AluInp.CURR_ALU_OUT
    u = UopConfig()
    u.enable_input(InpSel.SRC_0, 1)
    u.enable_input(InpSel.SRC_0_HI, 2)
    u.enable_input(InpSel.SRC_1, 3)
    u.enable_input(InpSel.SRC_1_HI, 4)
    u.enable_input(InpSel.ZERO, 5)
    d = u.datapath_config
    d[0].enable_alu(AluOp.MULTIPLY, D0, D2)
    d[0].pass_through_delay(1, 3, 4)
    d[1].enable_alu(AluOp.MULTIPLY, D1, D3)
    d[1].enable_delay_from_src(DelayInp.PREV_ALU_OUT, 0)
    d[1].pass_through_delay(4)
    d[2].enable_alu(AluOp.ADD, D0, PREV)
    d[2].pass_through_delay(4)
    if state == "steady":
        d[3].enable_alu(AluOp.ADD, CURR, PREV)
    elif state == "step":
        d[3].enable_alu(AluOp.ADD, D4, PREV)
    else:
        d[3].enable_alu(AluOp.BYPASS, D4, D4)
    for k in range(4, 8):
        d[k].pass_through_alu()
    return _seg_out(_seg_fsm(u, state), state)


def _segdot_1x(state):
    from concourse.dve_uop import AluInp, AluOp, InpSel, UopConfig

    D0, D1, D2 = (AluInp.PREV_DELAY_0, AluInp.PREV_DELAY_1,
                  AluInp.PREV_DELAY_2)
    PREV, CURR = AluInp.PREV_ALU_OUT, AluInp.CURR_ALU_OUT
    u = UopConfig()
    u.enable_input(InpSel.SRC_0, 1)
    u.enable_input(InpSel.SRC_1, 2)
    u.enable_input(InpSel.ZERO, 3)
    d = u.datapath_config
    d[0].enable_alu(AluOp.MULTIPLY, D0, D1)
    d[0].pass_through_delay(2)
    if state == "steady":
        d[1].enable_alu(AluOp.ADD, CURR, PREV)
    elif state == "step":
        d[1].enable_alu(AluOp.ADD, D2, PREV)
    else:
        d[1].enable_alu(AluOp.BYPASS, D2, D2)
    for k in range(2, 8):
        d[k].pass_through_alu()
    return _seg_out(_seg_fsm(u, state), state)


def _get_seg_ops():
    from concourse.dve_spec import Spec, Src0, Src1, scan, sq
    from concourse.dve_uop import AluOp

    def ref_p1(in0, in1, s0, s1, imm2):
        s = (in0.astype(np.float32) + in1.astype(np.float32)).sum(-1)
        return np.repeat(s[..., None], 2, axis=-1)

    def ref_sq(in0, in1, s0, s1, imm2):
        s = (in0.astype(np.float32) ** 2 + in1.astype(np.float32) ** 2).sum(-1)
        return np.repeat(s[..., None], 2, axis=-1)

    def ref_dot(in0, in1, s0, s1, imm2):
        s = (in0.astype(np.float32) * in1.astype(np.float32)).sum(-1)
        return np.repeat(s[..., None], 2, axis=-1)

    p1 = _register(
        "ANOVA_SEGP1", Spec(body=scan(AluOp.ADD, Src0 + Src1), reference=ref_p1),
        uops=_mk_uops(_segp1_1x), uops_2x=_mk_uops(_segp1_2x), subdim=True,
    )
    sq_ = _register(
        "ANOVA_SEGSQ",
        Spec(body=scan(AluOp.ADD, sq(Src0) + sq(Src1)), reference=ref_sq),
        uops=_mk_uops(_segsq_1x), uops_2x=_mk_uops(_segsq_2x), subdim=True,
    )
    dot = _register(
        "ANOVA_SEGDOT",
        Spec(body=scan(AluOp.ADD, Src0 * Src1), reference=ref_dot),
        uops=_mk_uops(_segdot_1x), uops_2x=_mk_uops(_segdot_2x), subdim=True,
    )
    return p1, sq_, dot


_ZW = [None]


def _get_zw_op():
    """w = (s0*p2 - p1^2) * p1 * s1 in one DVE pass (epilogue integrand)."""
    if _ZW[0] is not None:
        return _ZW[0]
    from concourse import dve_ops
    from concourse.dve_spec import C0, C1, Spec, Src0, Src1, lower, sq
    from concourse.dve_uop import DveOpSpec

    name = "ANOVA_ZW"
    for op in dve_ops.OPS:
        if op.name == name:
            _ZW[0] = op
            return op

    def _ref(in0, in1, s0, s1, imm2):
        p1 = in0.astype(np.float32)
        return (s0 * in1.astype(np.float32) - p1 * p1) * p1 * s1

    spec = Spec(body=(Src1 * C0 - sq(Src0)) * Src0 * C1, reference=_ref)
    row = 1 + len(dve_ops.OPS)
    shas = {}
    specs = {}
    for ver in ("v3", "v4"):
        dos = DveOpSpec(
            name=name, opcode=row, uops=lower(spec, ver=ver), rd1_en=True,
        )
        specs[ver] = dos
        shas[ver] = dos.sha(ver)
    op = dve_ops.DveOp(name, spec, subdim=False, uops_sha=shas)
    dve_ops.OPS.append(op)
    dve_ops.CUSTOM_DVE_SPECS[name] = spec
    dve_ops._SUB_OPCODE_FOR_NAME[name] = row
    for ver in ("v3", "v4"):
        dve_ops._COMPILE_CACHE[(name, ver)] = specs[ver]
    _ZW[0] = op
    return op


# ---------------------------------------------------------------- kernel


def build_nc(bp=_BP, n_qc=_N_QC):
    """Per-core Bass graph.

    Inputs:  "x"   [bp, 4096] fp16, free layout (h=f//32, d, f%32)
    Outputs: "out" [128, bp/128] f32, out[p, t] = y[t*128 + p]
    """
    from contextlib import ExitStack

    from concourse import bacc, mybir, tile

    f16 = mybir.dt.float16
    f32 = mybir.dt.float32
    AF = mybir.ActivationFunctionType
    OP = mybir.AluOpType
    AX = mybir.AxisListType

    segp1, segsq, segdot = _get_seg_ops()
    zw_op = _get_zw_op()

    T = bp // _P
    assert bp % _P == 0 and T % 2 == 0
    # qc tiles are the trailing tiles (so the ACT sin stream ends early),
    # but their DMA and DVE work are hoisted to the FRONT of the queues so
    # the extra Vector work lands in mid-stream slack, not on the tail
    qc_tiles = set(range(T - n_qc, T))
    qc_list = sorted(qc_tiles)

    nc = bacc.Bacc("TRN2", target_bir_lowering=False, debug=False)
    x_ext = nc.dram_tensor("x", [bp, _FD], f16, kind="ExternalInput").ap()
    y_ext = nc.dram_tensor("out", [_P, T], f32, kind="ExternalOutput").ap()

    with tile.TileContext(nc) as tc, ExitStack() as ctx:
        xp = ctx.enter_context(tc.tile_pool(name="x", bufs=7))
        sp = ctx.enter_context(tc.tile_pool(name="scr", bufs=2))
        pers = ctx.enter_context(tc.tile_pool(name="pers", bufs=1))

        # per-tile (value, value) fp16 pair buffers from the seg ops
        pb = pers.tile([_P, 2 * T * 2 * _D], f16, tag="pb")
        p1b = pb[:, :T * 2 * _D]            # (p1,p1) pairs, 128/tile
        p2b = pb[:, T * 2 * _D:]            # (p2,p2) pairs
        cb = pers.tile([_P, max(n_qc, 1) * 4 * _D], f16, tag="cb")
        sa1 = pers.tile([_P, T + 2], f32, tag="sa1")
        eacc = pers.tile([_P, T], f32, tag="eacc")
        p1f = pers.tile([_P, T], f32, tag="p1f")
        csum = pers.tile([_P, max(n_qc, 1)], f32, tag="csum")
        dq = pers.tile([_P, T], f32, tag="dq")
        out8 = pers.tile([_P, T], f32, tag="out8")
        er = pers.tile([_P, T * _D], f32, tag="er")

        xv_dram = x_ext.rearrange("(t p) q -> t p q", p=_P)

        # warm the Sin table during the first DMA wait (lazy load is ~2.6us)
        warm = pers.tile([_P, 1], f32, tag="warm")
        nc.gpsimd.memset(warm[:], 0.0)
        nc.scalar.activation(warm[:], warm[:], AF.Sin, scale=0.125)

        def seg(op, out_ap, in_lo, in_hi, pm=1):
            bi = nc.vector._custom_dve(
                op, out=out_ap,
                in0=in_lo.rearrange("p (s n) -> p s n", n=32),
                in1=in_hi.rearrange("p (s n) -> p s n", n=32),
            )
            bi.ins.perf_max = pm
            return bi

        def emit_heads(k, xt, nd, sacol, dcol):
            """Moment ops for a tile buffer xt holding nd d-groups (free
            nd*64, f-halves contiguous). Pair outputs land at column dcol
            (elements, 2 per d-group) of p1b/p2b (and cb for qc tiles)."""
            fd = nd * _F
            h = fd // 2
            xlo = xt[:, :h]
            xhi = xt[:, h:fd]
            seg(segp1, p1b[:, dcol:dcol + 2 * nd], xlo, xhi)
            seg(segsq, p2b[:, dcol:dcol + 2 * nd], xlo, xhi)
            if k in qc_tiles:
                # p3 on DVE with safe ops: wide x^2 via native 2x mul, then
                # a segmented dot sum(x^2 * x) per 32-seg, h-halves folded.
                qoff = qc_list.index(k) * 4 * _D
                x2t = sp.tile([_P, _FD], f16, tag="x2")
                nc.vector.tensor_mul(x2t[:, :fd], xt[:, :fd], xt[:, :fd])
                craw = cb[:, qoff:qoff + 4 * nd]
                bi = nc.vector._custom_dve(
                    segdot, out=craw,
                    in0=x2t[:, :fd].rearrange("p (s n) -> p s n", n=32),
                    in1=xt[:, :fd].rearrange("p (s n) -> p s n", n=32),
                )
                bi.ins.perf_max = 1
                nc.vector.tensor_add(
                    craw[:, :2 * nd], craw[:, :2 * nd], craw[:, 2 * nd:]
                )
            else:
                sscr = sp.tile([_P, _FD], f16, tag="sscr")
                nc.scalar.activation(
                    sscr[:, :fd], xt[:, :fd], AF.Sin, scale=0.125,
                    accum_out=sa1[:, sacol:sacol + 1],
                )

        def pairs_lo(buf, c0, c1):
            """[p, n, 1] view of the first element of each fp16 pair in
            buf[:, 2*c0 : 2*c1]."""
            return buf[:, 2 * c0:2 * c1].rearrange(
                "p (n two) -> p n two", two=2
            )[:, :, 0:1]

        def eruns(lo, hi):
            """Split [lo, hi) into kind-contiguous tile runs."""
            runs = []
            s = lo
            for c in range(lo + 1, hi + 1):
                if c == hi or (c in qc_tiles) != (s in qc_tiles):
                    runs.append((s, c))
                    s = c
            return runs

        def epilogue(c0, c1):
            """e3 combine for tile-columns [c0, c1) (uniform kind)."""
            if c0 >= c1:
                return
            is_qc = c0 in qc_tiles
            nt = c1 - c0
            d0, d1 = c0 * _D, c1 * _D
            nc.vector._custom_dve(
                zw_op, out=er[:, d0:d1],
                in0=pairs_lo(p1b, d0, d1),
                in1=pairs_lo(p2b, d0, d1),
                s0=3.0, s1=-1.0 / 6.0,
            )
            nc.vector.reduce_sum(
                eacc[:, c0:c1],
                er[:, d0:d1].rearrange("p (t d) -> p t d", t=nt, d=_D),
                axis=AX.X,
            )
            if is_qc:
                q0 = qc_list.index(c0)
                nc.vector.reduce_sum(
                    csum[:, q0:q0 + nt],
                    cb[:, 4 * q0 * _D:4 * q0 * _D + nt * 4 * _D].rearrange(
                        "p (t four) -> p t four", t=nt
                    )[:, :, :2 * _D],
                    axis=AX.X,
                )
                # out = eacc + (1/3) * (csum/2)   (pairs double-count c)
                nc.vector.scalar_tensor_tensor(
                    out8[:, c0:c1], csum[:, q0:q0 + nt], 1.0 / 6.0,
                    eacc[:, c0:c1], OP.mult, OP.add,
                )
            else:
                nc.vector.reduce_sum(
                    p1f[:, c0:c1],
                    pairs_lo(p1b, d0, d1).rearrange(
                        "p (t d) one -> p t (d one)", t=nt),
                    axis=AX.X,
                )
                # out = eacc + 128 p1f - 1024 S1   (P3 = 384 p1f - 3072 S1)
                nc.vector.scalar_tensor_tensor(
                    dq[:, c0:c1], sa1[:, c0:c1], -1024.0, eacc[:, c0:c1],
                    OP.mult, OP.add,
                )
                nc.vector.scalar_tensor_tensor(
                    out8[:, c0:c1], p1f[:, c0:c1], 128.0, dq[:, c0:c1],
                    OP.mult, OP.add,
                )

        # tile0 pieces first (unblocks both engines fastest)
        xta = sp.tile([_P, _H], f16, tag="xta")
        nc.sync.dma_start(xta[:, :_H // 2], xv_dram[0][:, :_H // 2])
        nc.sync.dma_start(xta[:, _H // 2:], xv_dram[0][:, _H // 2:_H])
        xtb = sp.tile([_P, _H], f16, tag="xtb")
        nc.sync.dma_start(xtb[:], xv_dram[0][:, _H:])
        # qc-tile DMA goes mid-stream: at the front it starves the early
        # sins, at the back its DVE burst extends the Vector tail (both
        # measured); mid-queue it overlaps the sin stream on both sides
        sins = [k for k in range(1, T) if k not in qc_tiles]
        order = sins[:3] + qc_list + sins[3:]
        xts = {}
        for k in order:
            xts[k] = xp.tile([_P, _FD], f16, tag="xt", name=f"xt{k}")
            nc.sync.dma_start(xts[k][:], xv_dram[k])

        emit_heads(0, xta[:, :_H // 2], _D // 4, T, 0)
        emit_heads(0, xta[:, _H // 2:], _D // 4, T + 1, 2 * (_D // 4))
        emit_heads(0, xtb, _D // 2, 0, 2 * (_D // 2))
        nc.vector.scalar_tensor_tensor(
            sa1[:, 0:1], sa1[:, T:T + 1], 1.0, sa1[:, 0:1],
            OP.mult, OP.add,
        )
        nc.vector.scalar_tensor_tensor(
            sa1[:, 0:1], sa1[:, T + 1:T + 2], 1.0, sa1[:, 0:1],
            OP.mult, OP.add,
        )
        for k in order:
            emit_heads(k, xts[k], _D, k, k * 2 * _D)
            if k == T // 2 + 1:
                for a, b in eruns(0, T // 2):
                    epilogue(a, b)
                nc.sync.dma_start(y_ext[:, :T // 2], out8[:, :T // 2])
        for a, b in eruns(T // 2, T):
            epilogue(a, b)
        nc.sync.dma_start(y_ext[:, T // 2:], out8[:, T // 2:])

    nc.compile()
    return nc


_nc_cache = {}


def _get_nc():
    key = (_BP, _N_QC)
    if key not in _nc_cache:
        _nc_cache[key] = build_nc(*key)
    return _nc_cache[key]


def _marshal(x: np.ndarray) -> list:
    """FULL fp32 input [B, F, D] -> per-core fp16 arrays [bp, 4096] in
    (tile-internal) layout: per batch, free = (h=f//32, d, f%32), with
    tile 0 of each core d-split into two self-contained halves."""
    x = np.asarray(x)
    assert x.shape == (_B, _F, _D), x.shape
    xc = x.reshape(_NCORES, _BP, _F, _D).astype(np.float16)
    xt = xc.reshape(_NCORES, _BP, 2, 32, _D).transpose(0, 1, 2, 4, 3)
    out = np.empty((_NCORES, _BP, _FD), dtype=np.float16)
    flat = xt.reshape(_NCORES, _BP, _FD)
    out[:, _P:] = flat[:, _P:]
    # tile 0 pieces: d 0-15, d 16-31 (quarters), d 32-63 (half); each piece
    # h-major so its f-halves are contiguous and self-contained
    t0 = xt[:, :_P]                                   # [c, 128, h, d, f2]
    pa = t0[:, :, :, 0:16].reshape(_NCORES, _P, 1024)
    pb = t0[:, :, :, 16:32].reshape(_NCORES, _P, 1024)
    pc = t0[:, :, :, 32:64].reshape(_NCORES, _P, 2048)
    out[:, :_P] = np.concatenate([pa, pb, pc], axis=2)
    return [np.ascontiguousarray(out[c]) for c in range(_NCORES)]


def kernel(x: np.ndarray) -> np.ndarray:
    from concourse.bass_utils import run_bass_kernel_spmd

    nc = _get_nc()
    shards = _marshal(x)
    in_maps = [{"x": shards[c]} for c in range(_NCORES)]
    res = run_bass_kernel_spmd(nc, in_maps, core_ids=list(range(_NCORES)))
    outs = []
    for c in range(_NCORES):
        o = res.results[c]["out"]  # [128, T]; o[p, t] = y[t*128 + p]
        outs.append(np.asarray(o).T.reshape(-1))
    return np.concatenate(outs).reshape(_B, 1).astype(np.float32)


# revision 25
# speedup vs baseline: 1.0410x; 1.0110x over previous
"""ANOVA-kernel (order 3) Trainium2 Bass kernel, v4 (segmented-scan DVE ops).

Math: per batch b, y[b] = sum_d e3(x[b, :, d]) with e3 the 3rd elementary
symmetric polynomial over F=64 fields. Newton's identities give

    e3 = (p1^3 - 3 p1 p2 + 2 p3) / 6,    p_k[b, d] = sum_f x[b, f, d]^k

Engine plan per [128 x 4096] fp16 tile (batch on partitions, free =
(h, d, f2) with h = f//32 major so both f-halves are contiguous):

  - p1[d], p2[d]: custom two-source segmented-scan DVE ops (SEGP1/SEGSQ)
    running in hardware 2X_1PORT mode: per cycle read 2 packed fp16 from
    each port (f-lo half on port0, f-hi on port1), accumulate in fp32,
    and write one (acc, acc) fp16 word per 32-element segment boundary
    (out_last_subdim). One 1127ns instruction replaces the old
    head-add + 5-level fold tree entirely.
  - p3: sin tiles use the Scalar engine (sum sin(x/8) = P1/8 - P3/3072 +
    O(t^5) with a free per-row accumulate); the last (qc) tile computes
    p3 on the DVE instead: wide x^2 via native 2x tensor_mul, then
    SEGDOT (a segmented dot accumulating sum(x^2 * x) per 32-segment,
    h-halves folded after). Cube-style segmented custom ops hang the
    DVE (hardware-bisected), so the dot form is used. The qc tile's DMA
    is placed mid-stream: at the front it starves the early sins, at
    the back its DVE burst extends the Vector tail (both measured).
  - epilogue: ZW custom op computes (3 p2 - p1^2) p1 / -6 per d from the
    strided pair buffers, reduce over d, then per-tile-kind combine.

Inputs are downcast to fp16 on the host (norm-rel error ~2e-3, far under
the 2e-2 gate), halving HBM traffic.

Sharding: pure data parallel over batch across 8 NeuronCores.
"""

import numpy as np

_B, _F, _D = 8192, 64, 64
_NCORES = 8
_BP = _B // _NCORES     # batches per core
_P = 128                # partitions per tile
_FD = _F * _D           # free elems per batch
_H = _FD // 2

_T = _BP // _P          # tiles per core (8)
_N_QC = 1               # trailing tiles computing p3 on DVE; rest use ACT sin

# ---------------------------------------------------------------- custom ops

_OPS = {}


def _register(name, spec, uops, uops_2x=None, subdim=False):
    from concourse import dve_ops
    from concourse.dve_uop import DveOpSpec

    if name in _OPS:
        return _OPS[name]
    for op in dve_ops.OPS:
        if op.name == name:
            _OPS[name] = op
            return op
    row = 1 + len(dve_ops.OPS)
    shas = {}
    specs = {}
    for ver in ("v3", "v4"):
        dos = DveOpSpec(
            name=name, opcode=row, uops=uops, rd1_en=True,
            uops_2x=uops_2x if ver == "v3" else None,
        )
        specs[ver] = dos
        shas[ver] = dos.sha(ver)
    op = dve_ops.DveOp(name, spec, subdim=subdim, uops_sha=shas)
    dve_ops.OPS.append(op)
    dve_ops.CUSTOM_DVE_SPECS[name] = spec
    dve_ops._SUB_OPCODE_FOR_NAME[name] = row
    for ver in ("v3", "v4"):
        dve_ops._COMPILE_CACHE[(name, ver)] = specs[ver]
    return op


def _seg_fsm(u, state):
    """3-state FSM [seed, steady, step] for segmented-scan ops."""
    from concourse.dve_uop import Trigger

    if state == "seed":
        u.require_inp0 = 0
        u.require_inp1 = 0
        u.trigger = (Trigger.COUNT, Trigger.NONE, Trigger.NONE)
        u.repeat_count = 1
        u.next_uop = (1, 0, 0)
    elif state == "steady":
        u.require_inp0 = 1
        u.require_inp1 = 1
        u.trigger = (Trigger.SRC_TENSOR_DONE, Trigger.SUB_DIM_DONE, Trigger.NONE)
        u.next_uop = (0, 2, 0)
    else:  # step
        u.require_inp0 = 1
        u.require_inp1 = 1
        u.trigger = (Trigger.SRC_TENSOR_DONE, Trigger.SUB_DIM_DONE, Trigger.COUNT)
        u.next_uop = (0, 2, 1)
        u.repeat_count = 1
    return u


def _seg_out(u, state):
    from concourse.dve_uop import OutPath, OutSel

    if state != "seed":
        # full-word (dup) boundary write: half-word subdim writes hang in
        # 2x mode (hardware-verified), so emit (acc, acc) pairs
        u.enable_output(OutSel.ALU_OUT, OutPath.WR0_LO)
        u.enable_output(OutSel.ALU_OUT, OutPath.WR0_HI)
        u.out_last_subdim_enable = 1
    return u


def _mk_uops(build_one):
    return [build_one(s) for s in ("seed", "steady", "step")]


def _segp1_2x(state):
    """acc[seg] += a0+a1+b0+b1 (4 fp16/cycle); (acc,acc) at each boundary."""
    from concourse.dve_uop import AluInp, AluOp, DelayInp, InpSel, UopConfig

    D0, D1, D2, D3, D4 = (AluInp.PREV_DELAY_0, AluInp.PREV_DELAY_1,
                          AluInp.PREV_DELAY_2, AluInp.PREV_DELAY_3,
                          AluInp.PREV_DELAY_4)
    PREV, CURR = AluInp.PREV_ALU_OUT, AluInp.CURR_ALU_OUT
    u = UopConfig()
    u.enable_input(InpSel.SRC_0, 1)
    u.enable_input(InpSel.SRC_0_HI, 2)
    u.enable_input(InpSel.SRC_1, 3)
    u.enable_input(InpSel.SRC_1_HI, 4)
    u.enable_input(InpSel.ZERO, 5)
    d = u.datapath_config
    d[0].enable_alu(AluOp.ADD, D0, D1)
    d[0].pass_through_delay(2, 3, 4)
    d[1].enable_alu(AluOp.ADD, D2, D3)
    d[1].enable_delay_from_src(DelayInp.PREV_ALU_OUT, 0)
    d[1].pass_through_delay(4)
    d[2].enable_alu(AluOp.ADD, D0, PREV)
    d[2].pass_through_delay(4)
    if state == "steady":
        d[3].enable_alu(AluOp.ADD, CURR, PREV)
    elif state == "step":
        d[3].enable_alu(AluOp.ADD, D4, PREV)
    else:
        d[3].enable_alu(AluOp.BYPASS, D4, D4)
    for k in range(4, 8):
        d[k].pass_through_alu()
    return _seg_out(_seg_fsm(u, state), state)


def _segp1_1x(state):
    from concourse.dve_uop import AluInp, AluOp, InpSel, UopConfig

    D0, D1, D2 = (AluInp.PREV_DELAY_0, AluInp.PREV_DELAY_1,
                  AluInp.PREV_DELAY_2)
    PREV, CURR = AluInp.PREV_ALU_OUT, AluInp.CURR_ALU_OUT
    u = UopConfig()
    u.enable_input(InpSel.SRC_0, 1)
    u.enable_input(InpSel.SRC_1, 2)
    u.enable_input(InpSel.ZERO, 3)
    d = u.datapath_config
    d[0].enable_alu(AluOp.ADD, D0, D1)
    d[0].pass_through_delay(2)
    if state == "steady":
        d[1].enable_alu(AluOp.ADD, CURR, PREV)
    elif state == "step":
        d[1].enable_alu(AluOp.ADD, D2, PREV)
    else:
        d[1].enable_alu(AluOp.BYPASS, D2, D2)
    for k in range(2, 8):
        d[k].pass_through_alu()
    return _seg_out(_seg_fsm(u, state), state)


def _segsq_2x(state):
    """acc[seg] += a0^2+a1^2+b0^2+b1^2 (4 fp16/cycle)."""
    from concourse.dve_uop import AluInp, AluOp, DelayInp, InpSel, UopConfig

    D0, D1, D2, D3, D4 = (AluInp.PREV_DELAY_0, AluInp.PREV_DELAY_1,
                          AluInp.PREV_DELAY_2, AluInp.PREV_DELAY_3,
                          AluInp.PREV_DELAY_4)
    PREV, CURR = AluInp.PREV_ALU_OUT, AluInp.CURR_ALU_OUT
    u = UopConfig()
    u.enable_input(InpSel.SRC_0, 1)
    u.enable_input(InpSel.SRC_0_HI, 2)
    u.enable_input(InpSel.SRC_1, 3)
    u.enable_input(InpSel.SRC_1_HI, 4)
    u.enable_input(InpSel.ZERO, 5)
    d = u.datapath_config
    d[0].enable_alu(AluOp.MULTIPLY, D0, D0)
    d[0].pass_through_delay(1, 2, 3, 4)
    d[1].enable_alu(AluOp.MULTIPLY, D1, D1)
    d[1].enable_delay_from_src(DelayInp.PREV_ALU_OUT, 0)
    d[1].pass_through_delay(2, 3, 4)
    d[2].enable_alu(AluOp.MULTIPLY, D2, D2)
    d[2].enable_delay_from_src(DelayInp.PREV_ALU_OUT, 1)
    d[2].pass_through_delay(0, 3, 4)
    d[3].enable_alu(AluOp.MULTIPLY, D3, D3)
    d[3].enable_delay_from_src(DelayInp.PREV_ALU_OUT, 2)
    d[3].pass_through_delay(0, 1, 4)
    d[4].enable_alu(AluOp.ADD, D0, D1)
    d[4].enable_delay_from_src(DelayInp.PREV_ALU_OUT, 3)
    d[4].pass_through_delay(2, 4)
    d[5].enable_alu(AluOp.ADD, D2, D3)
    d[5].enable_delay_from_src(DelayInp.PREV_ALU_OUT, 0)
    d[5].pass_through_delay(4)
    d[6].enable_alu(AluOp.ADD, D0, PREV)
    d[6].pass_through_delay(4)
    if state == "steady":
        d[7].enable_alu(AluOp.ADD, CURR, PREV)
    elif state == "step":
        d[7].enable_alu(AluOp.ADD, D4, PREV)
    else:
        d[7].enable_alu(AluOp.BYPASS, D4, D4)
    return _seg_out(_seg_fsm(u, state), state)


def _segsq_1x(state):
    from concourse.dve_uop import AluInp, AluOp, DelayInp, InpSel, UopConfig

    D0, D1, D2 = (AluInp.PREV_DELAY_0, AluInp.PREV_DELAY_1,
                  AluInp.PREV_DELAY_2)
    PREV, CURR = AluInp.PREV_ALU_OUT, AluInp.CURR_ALU_OUT
    u = UopConfig()
    u.enable_input(InpSel.SRC_0, 1)
    u.enable_input(InpSel.SRC_1, 2)
    u.enable_input(InpSel.ZERO, 3)
    d = u.datapath_config
    d[0].enable_alu(AluOp.MULTIPLY, D0, D0)
    d[0].pass_through_delay(1, 2)
    d[1].enable_alu(AluOp.MULTIPLY, D1, D1)
    d[1].enable_delay_from_src(DelayInp.PREV_ALU_OUT, 0)
    d[1].pass_through_delay(2)
    d[2].enable_alu(AluOp.ADD, D0, PREV)
    d[2].pass_through_delay(2)
    if state == "steady":
        d[3].enable_alu(AluOp.ADD, CURR, PREV)
    elif state == "step":
        d[3].enable_alu(AluOp.ADD, D2, PREV)
    else:
        d[3].enable_alu(AluOp.BYPASS, D2, D2)
    for k in range(4, 8):
        d[k].pass_through_alu()
    return _seg_out(_seg_fsm(u, state), state)


def _segc_1x(state):
    """acc[seg] += a^3 + b^3 (1+1 fp16/cycle, fp32 accum)."""
    from concourse.dve_uop import AluInp, AluOp, DelayInp, InpSel, UopConfig

    D0, D1, D2, D3 = (AluInp.PREV_DELAY_0, AluInp.PREV_DELAY_1,
                      AluInp.PREV_DELAY_2, AluInp.PREV_DELAY_3)
    PREV, CURR = AluInp.PREV_ALU_OUT, AluInp.CURR_ALU_OUT
    u = UopConfig()
    u.enable_input(InpSel.SRC_0, 1)
    u.enable_input(InpSel.SRC_1, 2)
    u.enable_input(InpSel.ZERO, 3)
    d = u.datapath_config
    d[0].enable_alu(AluOp.MULTIPLY, D0, D0)               # a^2
    d[0].pass_through_delay(0, 1, 2)
    d[1].enable_alu(AluOp.MULTIPLY, PREV, D0)             # a^3
    d[1].pass_through_delay(1, 2)
    d[2].enable_alu(AluOp.MULTIPLY, D1, D1)               # b^2
    d[2].enable_delay_from_src(DelayInp.PREV_ALU_OUT, 3)  # c3 <- a^3
    d[2].pass_through_delay(1, 2)
    d[3].enable_alu(AluOp.MULTIPLY, PREV, D1)             # b^3
    d[3].pass_through_delay(2, 3)
    d[4].enable_alu(AluOp.ADD, D3, PREV)                  # a^3 + b^3
    d[4].pass_through_delay(2)
    if state == "steady":
        d[5].enable_alu(AluOp.ADD, CURR, PREV)
    elif state == "step":
        d[5].enable_alu(AluOp.ADD, D2, PREV)
    else:
        d[5].enable_alu(AluOp.BYPASS, D2, D2)
    for k in range(6, 8):
        d[k].pass_through_alu()
    return _seg_out(_seg_fsm(u, state), state)


def _segdot_2x(state):
    """acc[seg] += u0*v0 + u1*v1 (u on port0, v on port1, packed fp16)."""
    from concourse.dve_uop import AluInp, AluOp, DelayInp, InpSel, UopConfig

    D0, D1, D2, D3, D4 = (AluInp.PREV_DELAY_0, AluInp.PREV_DELAY_1,
                          AluInp.PREV_DELAY_2, AluInp.PREV_DELAY_3,
                          AluInp.PREV_DELAY_4)
    PREV, CURR = AluInp.PREV_ALU_OUT, AluInp.CURR_ALU_OUT
    u = UopConfig()
    u.enable_input(InpSel.SRC_0, 1)
    u.enable_input(InpSel.SRC_0_HI, 2)
    u.enable_input(InpSel.SRC_1, 3)
    u.enable_input(InpSel.SRC_1_HI, 4)
    u.enable_input(InpSel.ZERO, 5)
    d = u.datapath_config
    d[0].enable_alu(AluOp.MULTIPLY, D0, D2)
    d[0].pass_through_delay(1, 3, 4)
    d[1].enable_alu(AluOp.MULTIPLY, D1, D3)
    d[1].enable_delay_from_src(DelayInp.PREV_ALU_OUT, 0)
    d[1].pass_through_delay(4)
    d[2].enable_alu(AluOp.ADD, D0, PREV)
    d[2].pass_through_delay(4)
    if state == "steady":
        d[3].enable_alu(AluOp.ADD, CURR, PREV)
    elif state == "step":
        d[3].enable_alu(AluOp.ADD, D4, PREV)
    else:
        d[3].enable_alu(AluOp.BYPASS, D4, D4)
    for k in range(4, 8):
        d[k].pass_through_alu()
    return _seg_out(_seg_fsm(u, state), state)


def _segdot_1x(state):
    from concourse.dve_uop import AluInp, AluOp, InpSel, UopConfig

    D0, D1, D2 = (AluInp.PREV_DELAY_0, AluInp.PREV_DELAY_1,
                  AluInp.PREV_DELAY_2)
    PREV, CURR = AluInp.PREV_ALU_OUT, AluInp.CURR_ALU_OUT
    u = UopConfig()
    u.enable_input(InpSel.SRC_0, 1)
    u.enable_input(InpSel.SRC_1, 2)
    u.enable_input(InpSel.ZERO, 3)
    d = u.datapath_config
    d[0].enable_alu(AluOp.MULTIPLY, D0, D1)
    d[0].pass_through_delay(2)
    if state == "steady":
        d[1].enable_alu(AluOp.ADD, CURR, PREV)
    elif state == "step":
        d[1].enable_alu(AluOp.ADD, D2, PREV)
    else:
        d[1].enable_alu(AluOp.BYPASS, D2, D2)
    for k in range(2, 8):
        d[k].pass_through_alu()
    return _seg_out(_seg_fsm(u, state), state)


def _get_seg_ops():
    from concourse.dve_spec import Spec, Src0, Src1, scan, sq
    from concourse.dve_uop import AluOp

    def ref_p1(in0, in1, s0, s1, imm2):
        s = (in0.astype(np.float32) + in1.astype(np.float32)).sum(-1)
        return np.repeat(s[..., None], 2, axis=-1)

    def ref_sq(in0, in1, s0, s1, imm2):
        s = (in0.astype(np.float32) ** 2 + in1.astype(np.float32) ** 2).sum(-1)
        return np.repeat(s[..., None], 2, axis=-1)

    def ref_dot(in0, in1, s0, s1, imm2):
        s = (in0.astype(np.float32) * in1.astype(np.float32)).sum(-1)
        return np.repeat(s[..., None], 2, axis=-1)

    p1 = _register(
        "ANOVA_SEGP1", Spec(body=scan(AluOp.ADD, Src0 + Src1), reference=ref_p1),
        uops=_mk_uops(_segp1_1x), uops_2x=_mk_uops(_segp1_2x), subdim=True,
    )
    sq_ = _register(
        "ANOVA_SEGSQ",
        Spec(body=scan(AluOp.ADD, sq(Src0) + sq(Src1)), reference=ref_sq),
        uops=_mk_uops(_segsq_1x), uops_2x=_mk_uops(_segsq_2x), subdim=True,
    )
    dot = _register(
        "ANOVA_SEGDOT",
        Spec(body=scan(AluOp.ADD, Src0 * Src1), reference=ref_dot),
        uops=_mk_uops(_segdot_1x), uops_2x=_mk_uops(_segdot_2x), subdim=True,
    )
    return p1, sq_, dot


_ZW = [None]


def _get_zw_op():
    """w = (s0*p2 - p1^2) * p1 * s1 in one DVE pass (epilogue integrand)."""
    if _ZW[0] is not None:
        return _ZW[0]
    from concourse import dve_ops
    from concourse.dve_spec import C0, C1, Spec, Src0, Src1, lower, sq
    from concourse.dve_uop import DveOpSpec

    name = "ANOVA_ZW"
    for op in dve_ops.OPS:
        if op.name == name:
            _ZW[0] = op
            return op

    def _ref(in0, in1, s0, s1, imm2):
        p1 = in0.astype(np.float32)
        return (s0 * in1.astype(np.float32) - p1 * p1) * p1 * s1

    spec = Spec(body=(Src1 * C0 - sq(Src0)) * Src0 * C1, reference=_ref)
    row = 1 + len(dve_ops.OPS)
    shas = {}
    specs = {}
    for ver in ("v3", "v4"):
        dos = DveOpSpec(
            name=name, opcode=row, uops=lower(spec, ver=ver), rd1_en=True,
        )
        specs[ver] = dos
        shas[ver] = dos.sha(ver)
    op = dve_ops.DveOp(name, spec, subdim=False, uops_sha=shas)
    dve_ops.OPS.append(op)
    dve_ops.CUSTOM_DVE_SPECS[name] = spec
    dve_ops._SUB_OPCODE_FOR_NAME[name] = row
    for ver in ("v3", "v4"):
        dve_ops._COMPILE_CACHE[(name, ver)] = specs[ver]
    _ZW[0] = op
    return op


# ---------------------------------------------------------------- kernel


def build_nc(bp=_BP, n_qc=_N_QC):
    """Per-core Bass graph.

    Inputs:  "x"   [bp, 4096] fp16, free layout (h=f//32, d, f%32)
    Outputs: "out" [128, bp/128] f32, out[p, t] = y[t*128 + p]
    """
    from contextlib import ExitStack

    from concourse import bacc, mybir, tile

    f16 = mybir.dt.float16
    f32 = mybir.dt.float32
    AF = mybir.ActivationFunctionType
    OP = mybir.AluOpType
    AX = mybir.AxisListType

    segp1, segsq, segdot = _get_seg_ops()
    zw_op = _get_zw_op()

    T = bp // _P
    assert bp % _P == 0 and T % 2 == 0
    # qc tiles are the trailing tiles (so the ACT sin stream ends early),
    # but their DMA and DVE work are hoisted to the FRONT of the queues so
    # the extra Vector work lands in mid-stream slack, not on the tail
    qc_tiles = set(range(T - n_qc, T))
    qc_list = sorted(qc_tiles)

    nc = bacc.Bacc("TRN2", target_bir_lowering=False, debug=False)
    x_ext = nc.dram_tensor("x", [bp, _FD], f16, kind="ExternalInput").ap()
    y_ext = nc.dram_tensor("out", [_P, T], f32, kind="ExternalOutput").ap()

    with tile.TileContext(nc) as tc, ExitStack() as ctx:
        xp = ctx.enter_context(tc.tile_pool(name="x", bufs=7))
        sp = ctx.enter_context(tc.tile_pool(name="scr", bufs=2))
        pers = ctx.enter_context(tc.tile_pool(name="pers", bufs=1))

        # per-tile (value, value) fp16 pair buffers from the seg ops
        pb = pers.tile([_P, 2 * T * 2 * _D], f16, tag="pb")
        p1b = pb[:, :T * 2 * _D]            # (p1,p1) pairs, 128/tile
        p2b = pb[:, T * 2 * _D:]            # (p2,p2) pairs
        cb = pers.tile([_P, max(n_qc, 1) * 4 * _D], f16, tag="cb")
        sa1 = pers.tile([_P, T + 2], f32, tag="sa1")
        eacc = pers.tile([_P, T], f32, tag="eacc")
        p1f = pers.tile([_P, T], f32, tag="p1f")
        csum = pers.tile([_P, max(n_qc, 1)], f32, tag="csum")
        dq = pers.tile([_P, T], f32, tag="dq")
        out8 = pers.tile([_P, T], f32, tag="out8")
        er = pers.tile([_P, T * _D], f32, tag="er")

        xv_dram = x_ext.rearrange("(t p) q -> t p q", p=_P)

        # warm the Sin table during the first DMA wait (lazy load is ~2.6us)
        warm = pers.tile([_P, 1], f32, tag="warm")
        nc.gpsimd.memset(warm[:], 0.0)
        nc.scalar.activation(warm[:], warm[:], AF.Sin, scale=0.125)

        def seg(op, out_ap, in_lo, in_hi, pm=1):
            bi = nc.vector._custom_dve(
                op, out=out_ap,
                in0=in_lo.rearrange("p (s n) -> p s n", n=32),
                in1=in_hi.rearrange("p (s n) -> p s n", n=32),
            )
            bi.ins.perf_max = pm
            return bi

        def emit_heads(k, xt, nd, sacol, dcol):
            """Moment ops for a tile buffer xt holding nd d-groups (free
            nd*64, f-halves contiguous). Pair outputs land at column dcol
            (elements, 2 per d-group) of p1b/p2b (and cb for qc tiles)."""
            fd = nd * _F
            h = fd // 2
            xlo = xt[:, :h]
            xhi = xt[:, h:fd]
            seg(segp1, p1b[:, dcol:dcol + 2 * nd], xlo, xhi)
            seg(segsq, p2b[:, dcol:dcol + 2 * nd], xlo, xhi)
            if k in qc_tiles:
                # p3 on DVE with safe ops: wide x^2 via native 2x mul, then
                # a segmented dot sum(x^2 * x) per 32-seg, h-halves folded.
                qoff = qc_list.index(k) * 4 * _D
                x2t = sp.tile([_P, _FD], f16, tag="x2")
                nc.vector.tensor_mul(x2t[:, :fd], xt[:, :fd], xt[:, :fd])
                craw = cb[:, qoff:qoff + 4 * nd]
                bi = nc.vector._custom_dve(
                    segdot, out=craw,
                    in0=x2t[:, :fd].rearrange("p (s n) -> p s n", n=32),
                    in1=xt[:, :fd].rearrange("p (s n) -> p s n", n=32),
                )
                bi.ins.perf_max = 1
                nc.vector.tensor_add(
                    craw[:, :2 * nd], craw[:, :2 * nd], craw[:, 2 * nd:]
                )
            else:
                sscr = sp.tile([_P, _FD], f16, tag="sscr")
                nc.scalar.activation(
                    sscr[:, :fd], xt[:, :fd], AF.Sin, scale=0.125,
                    accum_out=sa1[:, sacol:sacol + 1],
                )

        def pairs_lo(buf, c0, c1):
            """[p, n, 1] view of the first element of each fp16 pair in
            buf[:, 2*c0 : 2*c1]."""
            return buf[:, 2 * c0:2 * c1].rearrange(
                "p (n two) -> p n two", two=2
            )[:, :, 0:1]

        def eruns(lo, hi):
            """Split [lo, hi) into kind-contiguous tile runs."""
            runs = []
            s = lo
            for c in range(lo + 1, hi + 1):
                if c == hi or (c in qc_tiles) != (s in qc_tiles):
                    runs.append((s, c))
                    s = c
            return runs

        def epilogue(c0, c1):
            """e3 combine for tile-columns [c0, c1) (uniform kind)."""
            if c0 >= c1:
                return
            is_qc = c0 in qc_tiles
            nt = c1 - c0
            d0, d1 = c0 * _D, c1 * _D
            nc.vector._custom_dve(
                zw_op, out=er[:, d0:d1],
                in0=pairs_lo(p1b, d0, d1),
                in1=pairs_lo(p2b, d0, d1),
                s0=3.0, s1=-1.0 / 6.0,
            )
            nc.vector.reduce_sum(
                eacc[:, c0:c1],
                er[:, d0:d1].rearrange("p (t d) -> p t d", t=nt, d=_D),
                axis=AX.X,
            )
            if is_qc:
                q0 = qc_list.index(c0)
                nc.vector.reduce_sum(
                    csum[:, q0:q0 + nt],
                    cb[:, 4 * q0 * _D:4 * q0 * _D + nt * 4 * _D].rearrange(
                        "p (t four) -> p t four", t=nt
                    )[:, :, :2 * _D],
                    axis=AX.X,
                )
                # out = eacc + (1/3) * (csum/2)   (pairs double-count c)
                nc.vector.scalar_tensor_tensor(
                    out8[:, c0:c1], csum[:, q0:q0 + nt], 1.0 / 6.0,
                    eacc[:, c0:c1], OP.mult, OP.add,
                )
            else:
                nc.vector.reduce_sum(
                    p1f[:, c0:c1],
                    pairs_lo(p1b, d0, d1).rearrange(
                        "p (t d) one -> p t (d one)", t=nt),
                    axis=AX.X,
                )
                # out = eacc + 128 p1f - 1024 S1   (P3 = 384 p1f - 3072 S1)
                nc.vector.scalar_tensor_tensor(
                    dq[:, c0:c1], sa1[:, c0:c1], -1024.0, eacc[:, c0:c1],
                    OP.mult, OP.add,
                )
                nc.vector.scalar_tensor_tensor(
                    out8[:, c0:c1], p1f[:, c0:c1], 128.0, dq[:, c0:c1],
                    OP.mult, OP.add,
                )

        # tile0 pieces first (unblocks both engines fastest)
        xta = sp.tile([_P, _H], f16, tag="xta")
        nc.sync.dma_start(xta[:, :_H // 2], xv_dram[0][:, :_H // 2])
        nc.sync.dma_start(xta[:, _H // 2:], xv_dram[0][:, _H // 2:_H])
        xtb = sp.tile([_P, _H], f16, tag="xtb")
        nc.sync.dma_start(xtb[:], xv_dram[0][:, _H:])
        # qc-tile DMA goes mid-stream: at the front it starves the early
        # sins, at the back its DVE burst extends the Vector tail (both
        # measured); mid-queue it overlaps the sin stream on both sides
        sins = [k for k in range(1, T) if k not in qc_tiles]
        order = sins[:3] + qc_list + sins[3:]
        xts = {}
        for k in order:
            xts[k] = xp.tile([_P, _FD], f16, tag="xt", name=f"xt{k}")
            nc.sync.dma_start(xts[k][:], xv_dram[k])

        emit_heads(0, xta[:, :_H // 2], _D // 4, T, 0)
        emit_heads(0, xta[:, _H // 2:], _D // 4, T + 1, 2 * (_D // 4))
        emit_heads(0, xtb, _D // 2, 0, 2 * (_D // 2))
        nc.vector.scalar_tensor_tensor(
            sa1[:, 0:1], sa1[:, T:T + 1], 1.0, sa1[:, 0:1],
            OP.mult, OP.add,
        )
        nc.vector.scalar_tensor_tensor(
            sa1[:, 0:1], sa1[:, T + 1:T + 2], 1.0, sa1[:, 0:1],
            OP.mult, OP.add,
        )
        for k in order:
            emit_heads(k, xts[k], _D, k, k * 2 * _D)
            if k == T // 2 + 1:
                for a, b in eruns(0, T // 2):
                    epilogue(a, b)
                nc.sync.dma_start(y_ext[:, :T // 2], out8[:, :T // 2])
        for a, b in eruns(T // 2, T):
            epilogue(a, b)
        nc.sync.dma_start(y_ext[:, T // 2:], out8[:, T // 2:])

    nc.compile()
    return nc


_nc_cache = {}


def _get_nc():
    key = (_BP, _N_QC)
    if key not in _nc_cache:
        _nc_cache[key] = build_nc(*key)
    return _nc_cache[key]


def _marshal(x: np.ndarray) -> list:
    """FULL fp32 input [B, F, D] -> per-core fp16 arrays [bp, 4096] in
    (tile-internal) layout: per batch, free = (h=f//32, d, f%32), with
    tile 0 of each core d-split into two self-contained halves."""
    x = np.asarray(x)
    assert x.shape == (_B, _F, _D), x.shape
    xc = x.reshape(_NCORES, _BP, _F, _D).astype(np.float16)
    xt = xc.reshape(_NCORES, _BP, 2, 32, _D).transpose(0, 1, 2, 4, 3)
    out = np.empty((_NCORES, _BP, _FD), dtype=np.float16)
    flat = xt.reshape(_NCORES, _BP, _FD)
    out[:, _P:] = flat[:, _P:]
    # tile 0 pieces: d 0-15, d 16-31 (quarters), d 32-63 (half); each piece
    # h-major so its f-halves are contiguous and self-contained
    t0 = xt[:, :_P]                                   # [c, 128, h, d, f2]
    pa = t0[:, :, :, 0:16].reshape(_NCORES, _P, 1024)
    pb = t0[:, :, :, 16:32].reshape(_NCORES, _P, 1024)
    pc = t0[:, :, :, 32:64].reshape(_NCORES, _P, 2048)
    out[:, :_P] = np.concatenate([pa, pb, pc], axis=2)
    return [np.ascontiguousarray(out[c]) for c in range(_NCORES)]


def kernel(x: np.ndarray) -> np.ndarray:
    from concourse.bass_utils import run_bass_kernel_spmd

    nc = _get_nc()
    shards = _marshal(x)
    in_maps = [{"x": shards[c]} for c in range(_NCORES)]
    res = run_bass_kernel_spmd(nc, in_maps, core_ids=list(range(_NCORES)))
    outs = []
    for c in range(_NCORES):
        o = res.results[c]["out"]  # [128, T]; o[p, t] = y[t*128 + p]
        outs.append(np.asarray(o).T.reshape(-1))
    return np.concatenate(outs).reshape(_B, 1).astype(np.float32)
